# revision 27
# baseline (speedup 1.0000x reference)
"""Self-contained Trainium2 Bass kernel for MultiHeadAttention with QK-layernorm
and physical-coordinate RoPE.

Sharding: 8 cores = 4 batches x 2 head-groups (8 heads each).  Each core
computes its batch's projections for its head group, attention, and a partial
output projection (row-sharded Wo); the host sums the two partials per batch.

Host-side layout prep (free): x tensors pre-transposed to [DM, S]; q/k
projection weights permuted so each head's dims are [even(48) | odd(48)]
(rope pairs become contiguous halves; scores are invariant since q and k get
the same permutation) and augmented with one per-head column-sum column so
the matmul produces per-head sums (LN means) for free.

Device: one activation-table set (natural_log_exp: copy/square/ln/exp) after
a single trig load for precomputed rope tables; rstd = exp(-0.5*ln(var+eps)).
Projection eviction (stats -> rstd -> normalize -> rope -> transpose) is
software-pipelined across sq-tiles so the in-order engine queues overlap.
The attention y-matmul runs in fp8 DoubleRow (paired sk-tiles, 2x PE rate):
exp output and v are quantized to fp8e4 - both feed softmax-weighted
averages where quantization noise is suppressed by the participation ratio.
Out-proj contracts over 128-row repacked y chunks and interleaves with the
remaining attention heads per half.
"""

import math
import sys
import types

import numpy as np
import ml_dtypes

# ---- problem constants (hardcoded; kernel.py must not read spec/reference) ----
B, S, DM = 4, 2048, 1536
H_TOT, DH = 16, 96
HG = 8                      # heads per core
DV = HG * DH                # 768 per-core projection width
DVA = DV + HG               # + per-head sum columns
PHYS, NF = 3, 16            # phys dims, freqs
HF = PHYS * NF              # 48 rope pairs per head
MIN_LF, MAX_LF = -5.0, 3.0
LN_EPS = 1e-5
N_CORES = 8

SQ_TILES = S // 128         # 16
K_TILES = DM // 128         # 12
SCALE = 1.0 / math.sqrt(DH)
ESHIFT = -3.0               # exp(s*SCALE + ESHIFT): keep fp8 e4m3 in range
VPAD = 112                  # v row padded 97 -> 112 (DoubleRow step % 16)

# Cody-Waite 3-term split of 2*pi (c1/c2 have trailing mantissa zeroed so
# k*c1, k*c2 are exact in fp32 for small integer k)
def _cw_split():
    import struct
    def chop(x, bits):
        u = struct.unpack('<I', struct.pack('<f', np.float32(x)))[0]
        u &= ~((1 << bits) - 1)
        return struct.unpack('<f', struct.pack('<I', u))[0]
    two_pi = 2 * math.pi
    c1 = chop(two_pi, 12)
    c2 = chop(two_pi - c1, 12)
    c3 = np.float32(two_pi - c1 - c2)
    return float(c1), float(c2), float(c3)

CW1, CW2, CW3 = _cw_split()

_bf16 = ml_dtypes.bfloat16

# even/odd permutation within each head: [0,2,4,...,94, 1,3,...,95]
_PERM = np.concatenate([np.arange(0, DH, 2), np.arange(1, DH, 2)])


def _install_axon_hooks():
    """antenv.axon_hooks is absent on this image; shim it so trace=True works."""
    import antenv
    if hasattr(antenv, "axon_hooks"):
        return
    mod = types.ModuleType("antenv.axon_hooks")
    _hook = [None]
    mod.set_axon_ntff_profile_hook = lambda h: _hook.__setitem__(0, h)
    mod.get_axon_ntff_profile_hook = lambda: _hook[0]
    sys.modules["antenv.axon_hooks"] = mod
    antenv.axon_hooks = mod
    try:
        from trn_agent_boot.trn_boot import _ntff_profile_via_ctypes
        mod.set_axon_ntff_profile_hook(
            _ntff_profile_via_ctypes("/opt/axon/libaxon_pjrt.so"))
    except Exception:
        pass


def build_program(has_gamma, has_beta):
    from concourse import bacc
    import concourse.bass as bass
    import concourse.mybir as mybir
    import concourse.tile as tile
    from concourse.masks import make_identity
    from contextlib import ExitStack

    f32 = mybir.dt.float32
    bf = mybir.dt.bfloat16
    f8 = mybir.dt.float8e4
    AF = mybir.ActivationFunctionType
    ALU = mybir.AluOpType
    DR = mybir.MatmulPerfMode.DoubleRow

    nc = bacc.Bacc("TRN2", target_bir_lowering=False, debug=False,
                   num_devices=N_CORES)

    xqT = nc.dram_tensor("xqT", [DM, S], bf, kind="ExternalInput").ap()
    xkT = nc.dram_tensor("xkT", [DM, S], bf, kind="ExternalInput").ap()
    xvT = nc.dram_tensor("xvT", [DM, S], bf, kind="ExternalInput").ap()
    wqt = nc.dram_tensor("wqt", [DM, DVA], bf, kind="ExternalInput").ap()
    wkt = nc.dram_tensor("wkt", [DM, DVA], bf, kind="ExternalInput").ap()
    wvt = nc.dram_tensor("wvt", [DM, DV], bf, kind="ExternalInput").ap()
    wot = nc.dram_tensor("wot", [DV, DM], bf, kind="ExternalInput").ap()
    xq = nc.dram_tensor("xq", [S, PHYS], f32, kind="ExternalInput").ap()
    xk = nc.dram_tensor("xk", [S, PHYS], f32, kind="ExternalInput").ap()
    freqs = nc.dram_tensor("freqs", [1, NF], f32, kind="ExternalInput").ap()
    if has_gamma or has_beta:
        gbq = nc.dram_tensor("gbq", [2, DH], f32, kind="ExternalInput").ap()
        gbk = nc.dram_tensor("gbk", [2, DH], f32, kind="ExternalInput").ap()
    out = nc.dram_tensor("out", [S, DM], f32, kind="ExternalOutput").ap()
    out2 = nc.dram_tensor("out2", [S, DM], f32, kind="ExternalOutput").ap()

    out_t = out.rearrange("(t p) n -> p t n", p=128)       # [128, 16, 1536]
    out2_t = out2.rearrange("(t p) n -> p t n", p=128)
    xq_t = xq.rearrange("(t p) c -> p t c", p=128)         # [128, 16, 3]
    xk_t = xk.rearrange("(t p) c -> p t c", p=128)

    with tile.TileContext(nc) as tc, ExitStack() as ctx:
        consts = ctx.enter_context(tc.tile_pool(name="consts", bufs=1))

        ident = consts.tile([128, 128], bf, tag="ident")
        make_identity(nc, ident)

        freqs_sb = consts.tile([1, NF], f32, tag="freqs1")
        nc.sync.dma_start(out=freqs_sb, in_=freqs)
        freqs_bc = consts.tile([128, NF], f32, tag="freqsbc")
        nc.gpsimd.partition_broadcast(freqs_bc, freqs_sb)

        eps_sb = consts.tile([128, 1], f32, tag="eps")
        nc.vector.memset(eps_sb, LN_EPS)
        esh_sb = consts.tile([128, 1], f32, tag="esh")
        nc.vector.memset(esh_sb, ESHIFT)

        if has_gamma or has_beta:
            # gamma/beta broadcast to all partitions (permuted even/odd order)
            gbq_sb = consts.tile([1, 2, DH], f32, tag="gbq")
            nc.sync.dma_start(out=gbq_sb,
                              in_=gbq.rearrange("(o a) d -> o a d", o=1))
            gbk_sb = consts.tile([1, 2, DH], f32, tag="gbk")
            nc.sync.dma_start(out=gbk_sb,
                              in_=gbk.rearrange("(o a) d -> o a d", o=1))
            gb128 = consts.tile([128, 2, 2, DH], f32, tag="gb128")
            nc.gpsimd.partition_broadcast(
                gb128[:, 0].rearrange("p b d -> p (b d)"),
                gbq_sb.rearrange("o b d -> o (b d)"))
            nc.gpsimd.partition_broadcast(
                gb128[:, 1].rearrange("p b d -> p (b d)"),
                gbk_sb.rearrange("o b d -> o (b d)"))

        xq_sb = consts.tile([128, SQ_TILES, PHYS], f32, tag="xq")
        nc.sync.dma_start(out=xq_sb, in_=xq_t)
        xk_sb = consts.tile([128, SQ_TILES, PHYS], f32, tag="xk")
        nc.sync.dma_start(out=xk_sb, in_=xk_t)

        # persistent per-head activations
        heads = ctx.enter_context(tc.tile_pool(name="heads", bufs=1))
        qT_all = heads.tile([DH, HG, S], bf, tag="qT_all")
        kT_all = heads.tile([DH, HG, S], bf, tag="kT_all")
        # v with a leading ones column per head, padded to 128 cols so the
        # y-matmul stationary operand is a full 128-col weight (FWL)
        v_aug = heads.tile([128, SQ_TILES, HG, 128], bf, tag="v_aug")
        nc.vector.memset(v_aug[:, :, :, 0:1], 1.0)
        nc.vector.memset(v_aug[:, :, :, 1 + DH:], 0.0)
        # rope trig tables per tensor/tile: [cos|cos], [sin|sin] (x gamma)
        TBW = DH if has_gamma else HF
        CS = [heads.tile([128, SQ_TILES, TBW], bf, tag=f"CS{i}",
                         name=f"CS{i}") for i in (0, 1)]
        SN = [heads.tile([128, SQ_TILES, TBW], bf, tag=f"SN{i}",
                         name=f"SN{i}") for i in (0, 1)]
        BT = [heads.tile([128, SQ_TILES, DH], f32, tag=f"BT{i}",
              name=f"BT{i}") for i in (0, 1)] if has_beta else None

        with ExitStack() as proj_ctx:
            xT_pool = proj_ctx.enter_context(tc.tile_pool(name="xT", bufs=2))
            w_pool = proj_ctx.enter_context(tc.tile_pool(name="w", bufs=2))
            stat = proj_ctx.enter_context(tc.tile_pool(name="stat", bufs=4))
            rope = proj_ctx.enter_context(tc.tile_pool(name="rope", bufs=2))
            prew = proj_ctx.enter_context(tc.tile_pool(name="prew", bufs=2))
            ps_pool = proj_ctx.enter_context(
                tc.tile_pool(name="ps_proj", bufs=3, space="PSUM"))
            psT_pool = proj_ctx.enter_context(
                tc.tile_pool(name="ps_tp", bufs=2, space="PSUM"))

            # kick off the first weight/x DMAs before anything else so the
            # PE can start projecting while trig tables build
            w_q = w_pool.tile([128, K_TILES, DVA], bf, tag="w", name="w_q")
            nc.sync.dma_start(
                out=w_q, in_=wqt.rearrange("(j p) n -> p j n", p=128))
            SCH = 1024                   # s-columns per staged xT chunk
            xq0 = xT_pool.tile([128, K_TILES, SCH], bf, tag="xT", name="xq0")
            nc.sync.dma_start(
                out=xq0,
                in_=xqT.rearrange("(j p) s -> p j s", p=128)[:, :, 0:SCH])

            # ---------------- phase 0: rope trig tables ----------------
            MAGIC = 1.5 * 2.0 ** 23
            for ti in range(2):
                x_sb = xq_sb if ti == 0 else xk_sb
                for t in range(SQ_TILES):
                    theta = prew.tile([128, HF], f32, tag="theta")
                    for p in range(PHYS):
                        nc.vector.tensor_scalar_mul(
                            out=theta[:, p * NF:(p + 1) * NF], in0=freqs_bc,
                            scalar1=x_sb[:, t, p:p + 1])
                    # range-reduce for ACT Sin (valid domain [-pi, pi])
                    kmul = prew.tile([128, HF], f32, tag="kmul")
                    nc.vector.tensor_scalar(
                        out=kmul, in0=theta, scalar1=1.0 / (2 * math.pi),
                        scalar2=MAGIC, op0=ALU.mult, op1=ALU.add)
                    nc.vector.tensor_single_scalar(
                        out=kmul, in_=kmul, scalar=MAGIC, op=ALU.subtract)
                    nc.vector.cody_waite_cascade(out=theta, x=theta, k=kmul,
                                                 c1=CW1, c2=CW2, c3=CW3)
                    ts_ = kmul   # dead; reuse
                    tcs = prew.tile([128, HF], f32, tag="tcs")
                    nc.vector.add_range_wrap(out=ts_, in_=theta, shift=0.0,
                                             bound=math.pi, period=2 * math.pi)
                    nc.vector.add_range_wrap(out=tcs, in_=theta,
                                             shift=math.pi / 2,
                                             bound=math.pi, period=2 * math.pi)
                    if has_gamma:
                        cc = CS[ti][:, t].rearrange("p (a f) -> p a f", a=2)
                        ss = SN[ti][:, t].rearrange("p (a f) -> p a f", a=2)
                        cs48 = prew.tile([128, 2, HF], f32, tag="cs48")
                        nc.scalar.activation(out=cs48[:, 0, :], in_=tcs,
                                             func=AF.Sin, bias=0.0, scale=1.0)
                        nc.scalar.activation(out=cs48[:, 1, :], in_=ts_,
                                             func=AF.Sin, bias=0.0, scale=1.0)
                        cbc = cs48[:, 0:1, :].broadcast_to([128, 2, HF])
                        sbc = cs48[:, 1:2, :].broadcast_to([128, 2, HF])
                        g2 = gb128[:, ti, 0].rearrange("p (a f) -> p a f", a=2)
                        nc.vector.tensor_tensor(out=cc, in0=cbc, in1=g2,
                                                op=ALU.mult)
                        nc.vector.tensor_tensor(out=ss, in0=sbc, in1=g2,
                                                op=ALU.mult)
                    else:
                        nc.scalar.activation(out=CS[ti][:, t], in_=tcs,
                                             func=AF.Sin, bias=0.0, scale=1.0)
                        nc.scalar.activation(out=SN[ti][:, t], in_=ts_,
                                             func=AF.Sin, bias=0.0, scale=1.0)
                    if has_beta:
                        # rot(beta): [be*cos - bo*sin | be*sin + bo*cos]
                        be = gb128[:, ti, 1, 0:HF]
                        bo = gb128[:, ti, 1, HF:DH]
                        bt = BT[ti][:, t].rearrange("p (a f) -> p a f", a=2)
                        w0 = prew.tile([128, 2, HF], f32, tag="w0")
                        w1 = prew.tile([128, 2, HF], f32, tag="w1")
                        nc.vector.tensor_mul(out=w0[:, 0], in0=be,
                                             in1=cs48[:, 0])
                        nc.vector.tensor_mul(out=w0[:, 1], in0=be,
                                             in1=cs48[:, 1])
                        nc.vector.tensor_mul(out=w1[:, 0], in0=bo,
                                             in1=cs48[:, 1])
                        nc.vector.tensor_mul(out=w1[:, 1], in0=bo,
                                             in1=cs48[:, 0])
                        nc.vector.tensor_sub(out=bt[:, 0], in0=w0[:, 0],
                                             in1=w1[:, 0])
                        nc.vector.tensor_add(out=bt[:, 1], in0=w0[:, 1],
                                             in1=w1[:, 1])

            # ---------------- phase 1: projections (pipelined) --------------
            def stage_mm(ps, xT, tl, w_sb, wN):
                for j in range(K_TILES):
                    for c0, c1 in ((0, 512), (512, wN)):
                        nc.tensor.matmul(
                            ps[:, c0:c1],
                            lhsT=xT[:, j, tl * 128:(tl + 1) * 128],
                            rhs=w_sb[:, j, c0:c1],
                            start=(j == 0), stop=(j == K_TILES - 1))

            def stage_stats(ps):
                """psum -> mu, xraw(bf16), var; returns (xraw, mu, var)."""
                mu = stat.tile([128, HG], f32, tag="mu")
                nc.vector.tensor_single_scalar(out=mu, in_=ps[:, DV:DVA],
                                               scalar=1.0 / DH, op=ALU.mult)
                xraw = stat.tile([128, HG, DH], bf, tag="xraw")
                nc.scalar.copy(out=xraw.rearrange("p h d -> p (h d)"),
                               in_=ps[:, 0:DV])
                sq = stat.tile([128, DV], bf, tag="sq")
                nc.scalar.activation(out=sq, in_=ps[:, 0:DV], func=AF.Square,
                                     bias=0.0, scale=1.0)
                sumsq = stat.tile([128, HG], f32, tag="sumsq")
                nc.vector.tensor_reduce(
                    out=sumsq, in_=sq.rearrange("p (h d) -> p h d", d=DH),
                    axis=mybir.AxisListType.X, op=ALU.add)
                var = stat.tile([128, HG], f32, tag="var")
                nc.vector.tensor_mul(out=var, in0=mu, in1=mu)
                nc.vector.scalar_tensor_tensor(
                    out=var, in0=sumsq, scalar=1.0 / DH, in1=var,
                    op0=ALU.mult, op1=ALU.subtract)
                return xraw, mu, var

            def stage_rstd(var, mu):
                rstd = stat.tile([128, HG], f32, tag="rstd")
                nc.scalar.activation(out=rstd, in_=var, func=AF.Sqrt,
                                     bias=eps_sb, scale=1.0)
                nc.vector.reciprocal_approx_fast(out=rstd, in_=rstd)
                nmr = stat.tile([128, HG], f32, tag="nmr")
                nc.vector.scalar_tensor_tensor(
                    out=nmr, in0=mu, scalar=-1.0, in1=rstd,
                    op0=ALU.mult, op1=ALU.mult)
                return rstd, nmr

            def stage_rope_tp(ti, t, xraw, mu, rstd, nmr, dst_T):
                z = rope.tile([128, HG, DH], bf, tag="z")
                for h in range(4):
                    nc.vector.tensor_scalar(
                        out=z[:, h, :], in0=xraw[:, h, :],
                        scalar1=mu[:, h:h + 1], scalar2=rstd[:, h:h + 1],
                        op0=ALU.subtract, op1=ALU.mult)
                for h in range(4, HG):
                    nc.scalar.activation(
                        out=z[:, h, :], in_=xraw[:, h, :], func=AF.Identity,
                        bias=nmr[:, h:h + 1], scale=rstd[:, h:h + 1])
                t1 = rope.tile([128, HG, DH], bf, tag="t1")
                t2 = rope.tile([128, HG, DH], bf, tag="t2")
                if has_gamma:
                    ccb = CS[ti][:, t].rearrange(
                        "p (o d) -> p o d", o=1).broadcast_to([128, HG, DH])
                    ssb = SN[ti][:, t].rearrange(
                        "p (o d) -> p o d", o=1).broadcast_to([128, HG, DH])
                    nc.vector.tensor_mul(out=t1, in0=z, in1=ccb)
                    nc.vector.tensor_mul(out=t2, in0=z, in1=ssb)
                else:
                    ccb = CS[ti][:, t].rearrange(
                        "p (o a f) -> p o a f",
                        o=1, a=1).broadcast_to([128, HG, 2, HF])
                    ssb = SN[ti][:, t].rearrange(
                        "p (o a f) -> p o a f",
                        o=1, a=1).broadcast_to([128, HG, 2, HF])
                    z4 = z.rearrange("p h (a f) -> p h a f", a=2)
                    nc.vector.tensor_mul(
                        out=t1.rearrange("p h (a f) -> p h a f", a=2),
                        in0=z4, in1=ccb)
                    nc.vector.tensor_mul(
                        out=t2.rearrange("p h (a f) -> p h a f", a=2),
                        in0=z4, in1=ssb)
                rot = rope.tile([128, HG, DH], bf, tag="rot")
                nc.vector.tensor_sub(out=rot[:, :, 0:HF], in0=t1[:, :, 0:HF],
                                     in1=t2[:, :, HF:DH])
                nc.vector.tensor_add(out=rot[:, :, HF:DH], in0=t2[:, :, 0:HF],
                                     in1=t1[:, :, HF:DH])
                if has_beta:
                    btb = BT[ti][:, t].rearrange(
                        "p (o d) -> p o d", o=1).broadcast_to([128, HG, DH])
                    nc.vector.tensor_tensor(out=rot, in0=rot, in1=btb,
                                            op=ALU.add)
                for c in range(2):
                    tp = psT_pool.tile([DH, 4, 128], bf, tag="tp")
                    for i in range(4):
                        nc.tensor.transpose(out=tp[:, i, :],
                                            in_=rot[:, 4 * c + i, :],
                                            identity=ident)
                    nc.scalar.copy(
                        out=dst_T[:, 4 * c:4 * c + 4, t * 128:(t + 1) * 128],
                        in_=tp)

            specs = [(xvT, w_q, DV, None), (xqT, wqt, DVA, qT_all),
                     (xkT, wkt, DVA, kT_all)]
            for ti, (xT_dram, w_src, wN, dst_T) in enumerate(specs):
                if ti == 0:
                    w_sb = w_src
                else:
                    w_sb = w_pool.tile([128, K_TILES, wN], bf, tag="w",
                                       name=f"w{ti}")
                    dma_split(w_sb,
                              w_src.rearrange("(j p) n -> p j n", p=128))
                qk = ti - 1          # 0 for q, 1 for k; -1 for v
                xT_r = xT_dram.rearrange("(j p) s -> p j s", p=128)
                # software pipeline: A=matmul, B=stats, C=rstd, D=rope+tp
                live = {}
                for tt in range(SQ_TILES + 3):
                    if tt < SQ_TILES:
                        sc, tl = divmod(tt, SCH // 128)
                        if tl == 0:
                            if ti == 0 and sc == 0:
                                xT = xq0
                            else:
                                xT = xT_pool.tile([128, K_TILES, SCH],
                                                  bf, tag="xT")
                                dma_split(xT, xT_r[:, :, sc * SCH:
                                                   (sc + 1) * SCH])
                        ps = ps_pool.tile([128, DVA], f32, tag="proj")
                        stage_mm(ps, xT, tl, w_sb, wN)
                        live[tt] = [ps]
                    if ti >= 1:
                        if tt >= 1 and tt - 1 < SQ_TILES:
                            ps = live[tt - 1][0]
                            live[tt - 1] = list(stage_stats(ps))
                        if tt >= 2 and tt - 2 < SQ_TILES:
                            xraw, mu, var = live[tt - 2]
                            rstd, nmr = stage_rstd(var, mu)
                            live[tt - 2] = [xraw, mu, rstd, nmr]
                        if tt >= 3 and tt - 3 < SQ_TILES:
                            xraw, mu, rstd, nmr = live.pop(tt - 3)
                            stage_rope_tp(qk, tt - 3, xraw, mu, rstd, nmr,
                                          dst_T)
                    else:
                        if tt >= 1 and tt - 1 < SQ_TILES:
                            ps = live.pop(tt - 1)[0]
                            nc.scalar.copy(
                                out=v_aug[:, tt - 1, :, 1:1 + DH],
                                in_=ps[:, 0:DV].rearrange(
                                    "p (h d) -> p h d", d=DH))

        # ---------------- phase 2: attention + out-proj ----------------
        with ExitStack() as att_ctx:
            e_pool = att_ctx.enter_context(tc.tile_pool(name="E", bufs=4))
            s_pool = att_ctx.enter_context(
                tc.tile_pool(name="ps_s", bufs=2, space="PSUM"))
            y_pool = att_ctx.enter_context(
                tc.tile_pool(name="ps_y", bufs=1, space="PSUM"))
            o_pool = att_ctx.enter_context(
                tc.tile_pool(name="ps_o", bufs=2, space="PSUM"))
            nrm = att_ctx.enter_context(tc.tile_pool(name="nrm", bufs=2))
            oev = att_ctx.enter_context(tc.tile_pool(name="oev", bufs=6))
            wo_pool = att_ctx.enter_context(tc.tile_pool(name="wo", bufs=1))

            woT = wo_pool.tile([128, DV // 128, DM], bf, tag="woT")
            # normalized y repacked into 128-row contraction chunks: row
            # 96h+d of the 768-row y matrix lives at
            # [part (96h+d)%128, chunk (96h+d)//128]
            yNP = wo_pool.tile([128, DV // 128, S], bf, tag="yNP")
            nc.sync.dma_start(out=woT,
                              in_=wot.rearrange("(c p) n -> p c n", p=128))

            SH2 = S // 2
            SKP = SQ_TILES // 2          # sk pairs

            def outproj_tile(dst_t, c0, t):
                for n3 in range(3):
                    o_ps = o_pool.tile([128, 512], f32, tag="o", name="o_ps")
                    for kc in range(3):
                        nc.tensor.matmul(
                            o_ps,
                            lhsT=yNP[:, c0 + kc, t * 128:(t + 1) * 128],
                            rhs=woT[:, c0 + kc, n3 * 512:(n3 + 1) * 512],
                            start=(kc == 0), stop=(kc == 2))
                    o_sb = oev.tile([128, 512], f32, tag="osb", name="o_sb")
                    nc.vector.tensor_copy(out=o_sb, in_=o_ps)
                    nc.sync.dma_start(
                        out=dst_t[:, t, n3 * 512:(n3 + 1) * 512], in_=o_sb)

            def score_exp(h, half, sk):
                s_ps = s_pool.tile([128, 2, 512], f32, tag="S", name="s_ps")
                for i in range(2):
                    nc.tensor.matmul(
                        s_ps[:, i, :],
                        lhsT=kT_all[:, h, sk * 128:(sk + 1) * 128],
                        rhs=qT_all[:, h, half * SH2 + i * 512:
                                   half * SH2 + (i + 1) * 512],
                        start=True, stop=True)
                e = e_pool.tile([128, 2, 512], bf, tag="E", name="e")
                nc.scalar.activation(
                    out=e.rearrange("p a c -> p (a c)"),
                    in_=s_ps.rearrange("p a c -> p (a c)"),
                    func=AF.Exp, bias=0.0, scale=SCALE)
                return e

            def attn_unit(h, half, pre, nxt):
                """scores -> exp -> y accumulation, skewed one sk-tile.
                `pre` holds the first exp tiles prefetched by the previous
                unit; the next unit's first two score/exp pairs are emitted
                before this unit's normalize chain so the new unit opens
                with its y input already available."""
                y_ps = y_pool.tile([128, SH2], f32, tag="y", name="y_ps")
                y2 = y_ps.rearrange("p (a c) -> p a c", a=2)
                e_tiles = list(pre)
                for sk in range(SQ_TILES + 1):
                    if sk < SQ_TILES and sk >= len(pre):
                        e_tiles.append(score_exp(h, half, sk))
                    if sk >= 1:   # skew y one sk-tile behind scores
                        k0 = sk - 1
                        for i in range(2):
                            nc.tensor.matmul(
                                y2[:, i, :], lhsT=v_aug[:, k0, h, :],
                                rhs=e_tiles[k0][:, i, :],
                                start=(k0 == 0), stop=(k0 == SQ_TILES - 1))
                out_pre = []
                if nxt is not None:
                    for sk in range(2):
                        out_pre.append(score_exp(nxt[0], nxt[1], sk))
                # evacuate psum first (frees the y bank for the next unit),
                # then normalize by the ones-row denominator off-critical-path
                y_sb = nrm.tile([1 + DH, SH2], f32, tag="ysb")
                nc.vector.tensor_copy(out=y_sb[:, 0:512],
                                      in_=y_ps[0:1 + DH, 0:512])
                nc.scalar.copy(out=y_sb[:, 512:1024],
                               in_=y_ps[0:1 + DH, 512:1024])
                r1 = nrm.tile([1, SH2], f32, tag="r1")
                nc.vector.reciprocal_approx_fast(out=r1, in_=y_sb[0:1, :])
                rbc = nrm.tile([1 + DH, SH2], f32, tag="rbc")
                nc.gpsimd.partition_broadcast(rbc, r1)
                yst = nrm.tile([1 + DH, SH2], bf, tag="yst")
                nc.vector.tensor_tensor(out=yst, in0=y_sb, in1=rbc,
                                        op=ALU.mult)
                # scatter rows 1..96 into the repacked chunk layout (<=2 rects)
                g0 = DH * h
                d = 0
                while d < DH:
                    g = g0 + d
                    c, p = g // 128, g % 128
                    n = min(DH - d, 128 - p)
                    nc.sync.dma_start(
                        out=yNP[p:p + n, c, half * SH2:(half + 1) * SH2],
                        in_=yst[1 + d:1 + d + n, :])
                    d += n
                return out_pre

            # out-proj tile t only needs the half containing t, so jobs
            # unlock per (gating head, half) and interleave with later units
            ready = []
            units = [(h, half) for h in range(HG) for half in range(2)]
            pre = []
            for ui, (h, half) in enumerate(units):
                nxt = units[ui + 1] if ui + 1 < len(units) else None
                pre = attn_unit(h, half, pre, nxt)
                if h == 3:
                    ready += [(out_t, 0, t)
                              for t in range(8 * half, 8 * half + 8)]
                if h == 7:
                    ready += [(out2_t, 3, t)
                              for t in range(8 * half, 8 * half + 8)]
                rate = 5 if h >= 6 else 3
                for _ in range(min(rate, len(ready))):
                    outproj_tile(*ready.pop(0))
            while ready:
                outproj_tile(*ready.pop(0))

    nc.compile()
    return nc


_PROGRAMS = {}


def _get_program(has_gamma, has_beta):
    key = (has_gamma, has_beta)
    if key not in _PROGRAMS:
        _PROGRAMS[key] = build_program(has_gamma, has_beta)
    return _PROGRAMS[key]


def make_in_maps(qx, kx, vx, x_q, x_k, Wq, Wk, Wv, Wo, q_gamma, q_beta,
                 k_gamma, k_beta, has_gb):
    freqs = np.exp(np.linspace(MIN_LF, MAX_LF, NF)).astype(np.float32)

    def prep_w(W, rows):
        # [768, 1536] slice -> permute head dims even/odd -> transpose ->
        # append per-head sum columns -> [1536, 776] bf16
        Wg = W[rows].reshape(HG, DH, DM)[:, _PERM, :].reshape(DV, DM)
        WgT = np.ascontiguousarray(Wg.T).astype(_bf16)          # [1536, 768]
        sums = WgT.astype(np.float32).reshape(DM, HG, DH).sum(axis=2)
        return np.ascontiguousarray(
            np.concatenate([WgT.astype(np.float32), sums], axis=1)
        ).astype(_bf16)

    in_maps = []
    for core in range(N_CORES):
        b, g = core // 2, core % 2
        rows = slice(g * DV, (g + 1) * DV)
        m = {
            "xqT": np.ascontiguousarray(qx[b].T).astype(_bf16),
            "xkT": np.ascontiguousarray(kx[b].T).astype(_bf16),
            "xvT": np.ascontiguousarray(vx[b].T).astype(_bf16),
            "wqt": prep_w(Wq, rows),
            "wkt": prep_w(Wk, rows),
            "wvt": np.ascontiguousarray(Wv[rows].T).astype(_bf16),
            "wot": np.ascontiguousarray(Wo[:, rows].T).astype(_bf16),
            "xq": np.ascontiguousarray(x_q[b]).astype(np.float32),
            "xk": np.ascontiguousarray(x_k[b]).astype(np.float32),
            "freqs": freqs[None, :],
        }
        if has_gb:
            m["gbq"] = np.stack([q_gamma[_PERM], q_beta[_PERM]]).astype(
                np.float32)
            m["gbk"] = np.stack([k_gamma[_PERM], k_beta[_PERM]]).astype(
                np.float32)
        in_maps.append(m)
    return in_maps


LAST_EXEC_TIME_NS = None


def kernel(qx, kx, vx, x_q, x_k, Wq, Wk, Wv, Wo, q_gamma, q_beta,
           k_gamma, k_beta):
    global LAST_EXEC_TIME_NS
    import os
    _install_axon_hooks()
    from concourse.bass_utils import run_bass_kernel_spmd

    q_gamma = np.asarray(q_gamma); q_beta = np.asarray(q_beta)
    k_gamma = np.asarray(k_gamma); k_beta = np.asarray(k_beta)
    has_gamma = not (np.all(q_gamma == 1.0) and np.all(k_gamma == 1.0))
    has_beta = not (np.all(q_beta == 0.0) and np.all(k_beta == 0.0))

    nc = _get_program(has_gamma, has_beta)
    in_maps = make_in_maps(np.asarray(qx), np.asarray(kx), np.asarray(vx),
                           np.asarray(x_q), np.asarray(x_k), np.asarray(Wq),
                           np.asarray(Wk), np.asarray(Wv), np.asarray(Wo),
                           q_gamma, q_beta, k_gamma, k_beta,
                           has_gamma or has_beta)
    trace = bool(int(os.environ.get("KERNEL_TRACE", "0")))
    res = run_bass_kernel_spmd(nc, in_maps, list(range(N_CORES)), trace=trace)
    LAST_EXEC_TIME_NS = res.exec_time_ns
    outv = np.empty((B, S, DM), np.float32)
    for b in range(B):
        r0, r1 = res.results[2 * b], res.results[2 * b + 1]
        outv[b] = (r0["out"] + r0["out2"]) + (r1["out"] + r1["out2"])
    return outv
